# revision 19
# baseline (speedup 1.0000x reference)
"""Trainium2 Bass kernel for DecomposableAttentionEntailment.

Data-parallel over batch across 8 NeuronCores (16 batches/core).
All activations kept in transposed [feature, token] layout so every FF layer
is a direct PE contraction; att and att^T both computed by matmul operand
swap; global masked softmax done in two passes with ACT exp + accum_out.
Large matmuls run in fp32r (single-pass reduced-precision fp32, 4x faster
than fp32); producers write fp32r so walrus's rounding constraint is met.
"""
import sys

for p in ("/opt/trn_rl_repo",):
    if p not in sys.path:
        sys.path.insert(0, p)

import numpy as np

B, L, D, HW = 128, 512, 512, 512
N_CORES = 8
P = 128  # partitions

_cache = {}


def _build(bpc):
    import concourse.bass as bass
    import concourse.mybir as mybir
    from concourse import bacc, tile
    from concourse.masks import make_identity
    from contextlib import ExitStack

    f32 = mybir.dt.float32
    f32r = mybir.dt.float32r
    i32 = mybir.dt.int32
    Alu = mybir.AluOpType
    Act = mybir.ActivationFunctionType

    nc = bacc.Bacc("TRN2", target_bir_lowering=False, debug=False,
                   num_devices=N_CORES)

    prem = nc.declare_dram_parameter("premise", [bpc, L, D], f32, isOutput=False)
    hypo = nc.declare_dram_parameter("hypothesis", [bpc, L, D], f32, isOutput=False)
    pmask = nc.declare_dram_parameter("premise_mask", [bpc, L], i32, isOutput=False)
    hmask = nc.declare_dram_parameter("hypothesis_mask", [bpc, L], i32, isOutput=False)
    aw0 = nc.declare_dram_parameter("attend_w0", [D, HW], f32, isOutput=False)
    aw1 = nc.declare_dram_parameter("attend_w1", [HW, HW], f32, isOutput=False)
    cw0 = nc.declare_dram_parameter("compare_w0", [2 * D, HW], f32, isOutput=False)
    cw1 = nc.declare_dram_parameter("compare_w1", [HW, HW], f32, isOutput=False)
    gw0 = nc.declare_dram_parameter("aggregate_w0", [2 * HW, HW], f32, isOutput=False)
    gw1 = nc.declare_dram_parameter("aggregate_w1", [HW, HW], f32, isOutput=False)
    scw = nc.declare_dram_parameter("scorer", [HW, 2], f32, isOutput=False)
    out = nc.declare_dram_parameter("out", [bpc, 2], f32, isOutput=True)

    NEG = 1e9

    with tile.TileContext(nc) as tc, ExitStack() as octx:
        # ---------- outer pools ----------
        const_pool = octx.enter_context(tc.tile_pool(name="const", bufs=1))
        wpool = octx.enter_context(tc.tile_pool(name="weights", bufs=1))
        mpool = octx.enter_context(tc.tile_pool(name="masks", bufs=1))
        aggpool = octx.enter_context(tc.tile_pool(name="agg", bufs=1))
        smallp = octx.enter_context(tc.tile_pool(name="small", bufs=3))
        # PSUM pools: 4 + 2 + 2 banks
        psbig = octx.enter_context(tc.tile_pool(name="psbig", bufs=5, space="PSUM"))
        pstp = octx.enter_context(tc.tile_pool(name="pstp", bufs=2, space="PSUM"))
        pssm = octx.enter_context(tc.tile_pool(name="pssm", bufs=1, space="PSUM"))

        ident = const_pool.tile([P, P], f32)
        make_identity(nc, ident[:])
        ones_f = const_pool.tile([1, P], f32)
        nc.vector.memset(ones_f[:], 1.0)
        ones_row = const_pool.tile([1, P], f32r)
        nc.scalar.copy(ones_row[:], ones_f[:])
        identr = const_pool.tile([P, P], f32r)
        nc.scalar.copy(identr[:], ident[:])

        # ---------- masks preprocessing ----------
        # transposed masks [128, 4, bpc]: chunk r, partition = token within chunk
        pmT = mpool.tile([P, 4, bpc], f32)
        pmT_term = mpool.tile([P, 4, bpc], f32)
        hmT = mpool.tile([P, 4, bpc], f32)
        with tc.tile_pool(name="mtmp", bufs=1) as mtmp:
            pm_i = mtmp.tile([bpc, L], i32)
            nc.sync.dma_start(pm_i[:], pmask[:, :])
            hm_i = mtmp.tile([bpc, L], i32)
            nc.sync.dma_start(hm_i[:], hmask[:, :])
            pm_f = mtmp.tile([bpc, L], f32)
            nc.vector.tensor_copy(pm_f[:], pm_i[:])
            hm_f = mtmp.tile([bpc, L], f32)
            nc.vector.tensor_copy(hm_f[:], hm_i[:])
            for (mf, mt, mtt) in ((pm_f, pmT, pmT_term), (hm_f, hmT, None)):
                for r in range(4):
                    pst = pstp.tile([P, HW], f32, name="pst")
                    nc.tensor.transpose(pst[:, :bpc], mf[:, r * P:(r + 1) * P],
                                        ident[:bpc, :bpc])
                    nc.scalar.copy(mt[:, r, :], pst[:, :bpc])
                    if mtt is not None:
                        nc.vector.tensor_scalar(mtt[:, r, :], pst[:, :bpc],
                                                NEG, -NEG,
                                                op0=Alu.mult, op1=Alu.add)

        # aggregate staging: [128, 8, bpc]; chunks 0-3 = premise side f-chunks
        aggT = aggpool.tile([P, 8, bpc], f32)

        # ---------- batch loop (software-pipelined: front(b) || back(b-1)) ----
        with ExitStack() as bctx:
            nat_pool = bctx.enter_context(tc.tile_pool(name="natp", bufs=1))
            embp_pool = bctx.enter_context(tc.tile_pool(name="embp", bufs=2))
            embT_pool = bctx.enter_context(tc.tile_pool(name="embT", bufs=2))
            mid_pool = bctx.enter_context(tc.tile_pool(name="mid", bufs=1))
            proj_pool = bctx.enter_context(tc.tile_pool(name="proj", bufs=1))
            att_pool = bctx.enter_context(tc.tile_pool(name="attp", bufs=2))
            al_pool = bctx.enter_context(tc.tile_pool(name="alp", bufs=1))
            trm_pool = bctx.enter_context(tc.tile_pool(name="trmp", bufs=1))
            diag_pool = bctx.enter_context(tc.tile_pool(name="diagp", bufs=1))

            def emit_loads(b):
                hterm_i = trm_pool.tile([1, L], i32, name="hterm_i")
                nc.sync.dma_start(hterm_i[:], hmask[b:b + 1, :])
                nat_p = nat_pool.tile([P, 4, D], f32, name="nat_p")
                nat_h = nat_pool.tile([P, 4, D], f32, name="nat_h")
                for r in range(4):
                    nc.sync.dma_start(nat_p[:, r, :], prem[b, r * P:(r + 1) * P, :])
                    nc.sync.dma_start(nat_h[:, r, :], hypo[b, r * P:(r + 1) * P, :])
                return dict(hterm_i=hterm_i, nat_p=nat_p, nat_h=nat_h)

            def emit_front(b, ld):
                hterm = trm_pool.tile([1, L], f32r, name="hterm")
                nc.vector.tensor_scalar(hterm[:], ld["hterm_i"][:], NEG, -NEG,
                                        op0=Alu.mult, op1=Alu.add)

                emb_p = embp_pool.tile([P, 4, D], f32r, name="emb_p")
                emb_h = embp_pool.tile([P, 4, D], f32r, name="emb_h")
                nc.vector.tensor_copy(emb_p[:], ld["nat_p"][:])
                nc.vector.tensor_copy(emb_h[:], ld["nat_h"][:])
                # diag(mask) per token chunk: transposing matmul applies the mask
                diag_p = diag_pool.tile([P, 4, P], f32r, name="diag_p")
                diag_h = diag_pool.tile([P, 4, P], f32r, name="diag_h")
                for r in range(4):
                    nc.vector.tensor_scalar_mul(diag_p[:, r, :], identr[:],
                                                pmT[:, r, b:b + 1])
                    nc.vector.tensor_scalar_mul(diag_h[:, r, :], identr[:],
                                                hmT[:, r, b:b + 1])
                embT_p = embT_pool.tile([P, 4, L], f32r, name="embT_p")
                embT_h = embT_pool.tile([P, 4, L], f32r, name="embT_h")
                for (emb, dg, dst) in ((emb_p, diag_p, embT_p),
                                       (emb_h, diag_h, embT_h)):
                    for rd in range(4):
                        pst = pstp.tile([P, HW], f32, name="pst")
                        for rc in range(4):
                            nc.tensor.matmul(
                                pst[:, rc * P:(rc + 1) * P],
                                emb[:, rc, rd * P:(rd + 1) * P],
                                dg[:, rc, :], start=True, stop=True)
                        nc.vector.tensor_copy(dst[:, rd, :], pst[:])

                # proj FF (attend)
                projs = []
                for (embT, nm) in ((embT_p, "projT_p"), (embT_h, "projT_h")):
                    t0T = mid_pool.tile([P, 4, L], f32r, name="midT", tag="midT")
                    for ft in range(4):
                        ps = psbig.tile([P, HW], f32, name="psb")
                        for c in range(4):
                            nc.tensor.matmul(ps[:], aw0_sb[:, c, ft * P:(ft + 1) * P],
                                             embT[:, c, :],
                                             start=(c == 0), stop=(c == 3))
                        nc.scalar.activation(t0T[:, ft, :], ps[:], Act.Relu)
                    projT = proj_pool.tile([P, 4, L], f32r, name=nm)
                    for ft in range(4):
                        ps = psbig.tile([P, HW], f32, name="psb")
                        for c in range(4):
                            nc.tensor.matmul(ps[:], aw1_sb[:, c, ft * P:(ft + 1) * P],
                                             t0T[:, c, :],
                                             start=(c == 0), stop=(c == 3))
                        nc.scalar.activation(projT[:, ft, :], ps[:], Act.Relu)
                    projs.append(projT)
                projT_p, projT_h = projs

                # att [p,h] and attT [h,p] with masking
                att = att_pool.tile([P, 4, L], f32r, name="att")
                mx = smallp.tile([P, 4], f32, name="mx")
                for pt in range(4):
                    ps = psbig.tile([P, HW], f32, name="psb")
                    for c in range(4):
                        nc.tensor.matmul(ps[:], projT_p[:, c, pt * P:(pt + 1) * P],
                                         projT_h[:, c, :],
                                         start=(c == 0), stop=False)
                    nc.tensor.matmul(ps[:], ones_row[:1, :], hterm[0:1, :],
                                     start=False, stop=True)
                    nc.scalar.activation(att[:, pt, :], ps[:], Act.Identity,
                                         bias=pmT_term[:, pt, b:b + 1])
                    nc.vector.reduce_max(mx[:, pt:pt + 1],
                                         att[:, pt, :].bitcast(f32),
                                         axis=mybir.AxisListType.X)
                # global max -> -gmax broadcast
                m1 = smallp.tile([P, 1], f32, name="m1")
                nc.vector.reduce_max(m1[:], mx[:], axis=mybir.AxisListType.X)
                pstm = pssm.tile([P, P], f32, name="pstm")
                nc.tensor.transpose(pstm[:1, :], m1[:], ident[:])
                gmax = smallp.tile([1, 1], f32, name="gmax")
                nc.vector.reduce_max(gmax[:], pstm[:1, :],
                                     axis=mybir.AxisListType.X)
                psb1 = pssm.tile([P, P], f32, name="pstm")
                nc.tensor.matmul(psb1[:, :1], ones_f[:1, :], gmax[:1, :],
                                 start=True, stop=True)
                negmax = smallp.tile([P, 1], f32, name="negmax")
                nc.scalar.mul(negmax[:], psb1[:, :1], -1.0)

                # exp (one big AP per tensor; accum_out gives Z row-sums)
                s1 = smallp.tile([P, 1], f32, name="s1")
                nc.scalar.activation(att[:, :, :], att[:, :, :], Act.Exp,
                                     bias=negmax[:, 0:1], accum_out=s1[:])
                pstz = pssm.tile([P, P], f32, name="pstm")
                nc.tensor.transpose(pstz[:1, :], s1[:], ident[:])
                zsum = smallp.tile([1, 1], f32, name="zsum")
                nc.vector.reduce_sum(zsum[:], pstz[:1, :],
                                     axis=mybir.AxisListType.X)
                rz = smallp.tile([1, 1], f32, name="rz")
                nc.vector.reciprocal(rz[:], zsum[:])
                psb2 = pssm.tile([P, P], f32, name="pstm")
                nc.tensor.matmul(psb2[:, :1], ones_f[:1, :], rz[:1, :],
                                 start=True, stop=True)
                rzb = smallp.tile([P, 1], f32, name="rzb")
                nc.scalar.copy(rzb[:], psb2[:, :1])
                return dict(b=b, emb_p=emb_p, emb_h=emb_h, embT_p=embT_p,
                            embT_h=embT_h, att=att, rzb=rzb)

            def emit_back(s):
                b = s["b"]
                # S^T via PE block transposes of the exp'd S (masking inherited)
                attT = att_pool.tile([P, 4, L], f32r, name="attT", bufs=1)
                for hi in range(4):
                    pst = pstp.tile([P, HW], f32, name="pst")
                    for pj in range(4):
                        nc.tensor.matmul(
                            pst[:, pj * P:(pj + 1) * P],
                            s["att"][:, pj, hi * P:(hi + 1) * P],
                            identr[:], start=True, stop=True)
                    nc.vector.tensor_copy(attT[:, hi, :], pst[:])
                # alignments; 1/Z normalization fused into PSUM evacuation
                alT_p = al_pool.tile([P, 4, L], f32r, name="alT_p")
                for dt in range(4):
                    ps = psbig.tile([P, HW], f32, name="psb")
                    for c in range(4):
                        nc.tensor.matmul(ps[:],
                                         s["emb_h"][:, c, dt * P:(dt + 1) * P],
                                         attT[:, c, :],
                                         start=(c == 0), stop=(c == 3))
                    nc.vector.tensor_scalar_mul(alT_p[:, dt, :], ps[:],
                                                s["rzb"][:, 0:1])
                alT_h = al_pool.tile([P, 4, L], f32r, name="alT_h")
                for dt in range(4):
                    ps = psbig.tile([P, HW], f32, name="psb")
                    for c in range(4):
                        nc.tensor.matmul(ps[:],
                                         s["emb_p"][:, c, dt * P:(dt + 1) * P],
                                         s["att"][:, c, :],
                                         start=(c == 0), stop=(c == 3))
                    nc.vector.tensor_scalar_mul(alT_h[:, dt, :], ps[:],
                                                s["rzb"][:, 0:1])

                # compare FF + aggregate row-sums
                for (embT, alT, coff) in ((s["embT_p"], alT_p, 0),
                                          (s["embT_h"], alT_h, 4)):
                    c1T = mid_pool.tile([P, 4, L], f32r, name="midT", tag="midT")
                    for ft in range(4):
                        ps = psbig.tile([P, HW], f32, name="psb")
                        for c in range(4):
                            nc.tensor.matmul(ps[:],
                                             cw0_sb[:, c, ft * P:(ft + 1) * P],
                                             embT[:, c, :],
                                             start=(c == 0), stop=False)
                        for c in range(4):
                            nc.tensor.matmul(ps[:],
                                             cw0_sb[:, 4 + c, ft * P:(ft + 1) * P],
                                             alT[:, c, :],
                                             start=False, stop=(c == 3))
                        nc.vector.tensor_scalar_max(c1T[:, ft, :], ps[:], 0.0)
                    for ft in range(4):
                        ps = psbig.tile([P, HW], f32, name="psb")
                        for c in range(4):
                            nc.tensor.matmul(ps[:],
                                             cw1_sb[:, c, ft * P:(ft + 1) * P],
                                             c1T[:, c, :],
                                             start=(c == 0), stop=(c == 3))
                        # scratch relu output overwrites the dying alT slot
                        nc.scalar.activation(alT[:, ft, :], ps[:], Act.Relu,
                                             accum_out=aggT[:, coff + ft, b:b + 1])

            ld0 = emit_loads(0)

            # weights (after batch-0 loads so its DMAs queue first):
            # DMA to f32 staging borrowed from att_pool slots (reused by the
            # batch loop afterwards), then ACT copy rounds into f32r
            def load_w(name, src, chunks):
                t = wpool.tile([P, chunks, HW], f32r, name=name)
                for h in range(0, chunks, 4):
                    n = min(4, chunks - h)
                    stage = att_pool.tile([P, 4, HW], f32, name="att", tag="att")
                    nc.sync.dma_start(
                        stage[:, :n, :],
                        src[h * P:(h + n) * P].rearrange("(c p) f -> p c f", p=P))
                    nc.scalar.copy(t[:, h:h + n, :], stage[:, :n, :])
                return t

            aw0_sb = load_w("aw0_sb", aw0, 4)
            aw1_sb = load_w("aw1_sb", aw1, 4)
            cw0_sb = load_w("cw0_sb", cw0, 8)
            cw1_sb = load_w("cw1_sb", cw1, 4)

            prev = None
            ld = ld0
            for b in range(bpc):
                nld = emit_loads(b + 1) if b + 1 < bpc else None
                st = emit_front(b, ld)
                if prev is not None:
                    emit_back(prev)
                prev = st
                ld = nld
            emit_back(prev)

        # ---------- final aggregate FF + scorer + softmax (plain fp32) ----------
        with ExitStack() as fctx:
            fpool = fctx.enter_context(tc.tile_pool(name="fin", bufs=1))
            gw0_sb = fpool.tile([P, 8, HW], f32)
            nc.sync.dma_start(gw0_sb[:], gw0[:].rearrange("(c p) f -> p c f", p=P))
            gw1_sb = fpool.tile([P, 4, HW], f32)
            nc.sync.dma_start(gw1_sb[:], gw1[:].rearrange("(c p) f -> p c f", p=P))
            scw_sb = fpool.tile([P, 4, 2], f32)
            nc.sync.dma_start(scw_sb[:], scw[:].rearrange("(c p) s -> p c s", p=P))

            h1T = fpool.tile([P, 4, bpc], f32)
            for ft in range(4):
                ps = psbig.tile([P, HW], f32, name="psb")
                for c in range(8):
                    nc.tensor.matmul(ps[:, :bpc], gw0_sb[:, c, ft * P:(ft + 1) * P],
                                     aggT[:, c, :], start=(c == 0), stop=(c == 7))
                nc.scalar.activation(h1T[:, ft, :], ps[:, :bpc], Act.Relu)
            h2T = fpool.tile([P, 4, bpc], f32)
            for ft in range(4):
                ps = psbig.tile([P, HW], f32, name="psb")
                for c in range(4):
                    nc.tensor.matmul(ps[:, :bpc], gw1_sb[:, c, ft * P:(ft + 1) * P],
                                     h1T[:, c, :], start=(c == 0), stop=(c == 3))
                nc.scalar.activation(h2T[:, ft, :], ps[:, :bpc], Act.Relu)
            ps2 = pssm.tile([P, P], f32, name="pstm")
            for c in range(4):
                nc.tensor.matmul(ps2[:bpc, :2], h2T[:, c, :], scw_sb[:, c, :],
                                 start=(c == 0), stop=(c == 3))
            mx2 = fpool.tile([bpc, 1], f32)
            nc.vector.reduce_max(mx2[:], ps2[:bpc, :2], axis=mybir.AxisListType.X)
            negm2 = fpool.tile([bpc, 1], f32)
            nc.vector.tensor_scalar_mul(negm2[:], mx2[:], -1.0)
            prob = fpool.tile([bpc, 2], f32)
            zs2 = fpool.tile([bpc, 1], f32)
            nc.scalar.activation(prob[:], ps2[:bpc, :2], Act.Exp,
                                 bias=negm2[:, 0:1], accum_out=zs2[:])
            rz2 = fpool.tile([bpc, 1], f32)
            nc.vector.reciprocal(rz2[:], zs2[:])
            nc.vector.tensor_scalar_mul(prob[:], prob[:], rz2[:, 0:1])
            nc.sync.dma_start(out[:, :], prob[:])

    nc.compile()
    return nc


def _get(bpc):
    if bpc not in _cache:
        _cache[bpc] = _build(bpc)
    return _cache[bpc]


def kernel(premise, hypothesis, premise_mask, hypothesis_mask,
           attend_w0, attend_w1, compare_w0, compare_w1,
           aggregate_w0, aggregate_w1, scorer):
    from concourse.bass_utils import run_bass_kernel_spmd

    premise = np.asarray(premise, dtype=np.float32)
    hypothesis = np.asarray(hypothesis, dtype=np.float32)
    premise_mask = np.asarray(premise_mask, dtype=np.int32)
    hypothesis_mask = np.asarray(hypothesis_mask, dtype=np.int32)
    weights = {
        "attend_w0": np.asarray(attend_w0, dtype=np.float32),
        "attend_w1": np.asarray(attend_w1, dtype=np.float32),
        "compare_w0": np.asarray(compare_w0, dtype=np.float32),
        "compare_w1": np.asarray(compare_w1, dtype=np.float32),
        "aggregate_w0": np.asarray(aggregate_w0, dtype=np.float32),
        "aggregate_w1": np.asarray(aggregate_w1, dtype=np.float32),
        "scorer": np.asarray(scorer, dtype=np.float32),
    }
    bn = premise.shape[0]
    bpc = bn // N_CORES
    nc = _get(bpc)
    in_maps = []
    for c in range(N_CORES):
        sl = slice(c * bpc, (c + 1) * bpc)
        m = {
            "premise": np.ascontiguousarray(premise[sl]),
            "hypothesis": np.ascontiguousarray(hypothesis[sl]),
            "premise_mask": np.ascontiguousarray(premise_mask[sl]),
            "hypothesis_mask": np.ascontiguousarray(hypothesis_mask[sl]),
        }
        m.update(weights)
        in_maps.append(m)
    global _last_in_maps
    _last_in_maps = in_maps
    res = run_bass_kernel_spmd(nc, in_maps, core_ids=list(range(N_CORES)))
    return np.concatenate([res.results[c]["out"] for c in range(N_CORES)],
                          axis=0).astype(np.float32)


# revision 20
# speedup vs baseline: 1.0819x; 1.0819x over previous
"""Trainium2 Bass kernel for DecomposableAttentionEntailment.

Data-parallel over batch across 8 NeuronCores (16 batches/core).
All activations kept in transposed [feature, token] layout so every FF layer
is a direct PE contraction; att and att^T both computed by matmul operand
swap; global masked softmax done in two passes with ACT exp + accum_out.
Large matmuls run in fp32r (single-pass reduced-precision fp32, 4x faster
than fp32); producers write fp32r so walrus's rounding constraint is met.
"""
import sys

for p in ("/opt/trn_rl_repo",):
    if p not in sys.path:
        sys.path.insert(0, p)

import numpy as np

B, L, D, HW = 128, 512, 512, 512
N_CORES = 8
P = 128  # partitions

_cache = {}


def _build(bpc):
    import concourse.bass as bass
    import concourse.mybir as mybir
    from concourse import bacc, tile
    from concourse.masks import make_identity
    from contextlib import ExitStack

    f32 = mybir.dt.float32
    f32r = mybir.dt.float32r
    i32 = mybir.dt.int32
    Alu = mybir.AluOpType
    Act = mybir.ActivationFunctionType

    nc = bacc.Bacc("TRN2", target_bir_lowering=False, debug=False,
                   num_devices=N_CORES)

    prem = nc.declare_dram_parameter("premise", [bpc, L, D], f32, isOutput=False)
    hypo = nc.declare_dram_parameter("hypothesis", [bpc, L, D], f32, isOutput=False)
    pmask = nc.declare_dram_parameter("premise_mask", [bpc, L], i32, isOutput=False)
    hmask = nc.declare_dram_parameter("hypothesis_mask", [bpc, L], i32, isOutput=False)
    aw0 = nc.declare_dram_parameter("attend_w0", [D, HW], f32, isOutput=False)
    aw1 = nc.declare_dram_parameter("attend_w1", [HW, HW], f32, isOutput=False)
    cw0 = nc.declare_dram_parameter("compare_w0", [2 * D, HW], f32, isOutput=False)
    cw1 = nc.declare_dram_parameter("compare_w1", [HW, HW], f32, isOutput=False)
    gw0 = nc.declare_dram_parameter("aggregate_w0", [2 * HW, HW], f32, isOutput=False)
    gw1 = nc.declare_dram_parameter("aggregate_w1", [HW, HW], f32, isOutput=False)
    scw = nc.declare_dram_parameter("scorer", [HW, 2], f32, isOutput=False)
    out = nc.declare_dram_parameter("out", [bpc, 2], f32, isOutput=True)

    NEG = 1e9

    with tile.TileContext(nc) as tc, ExitStack() as octx:
        # ---------- outer pools ----------
        const_pool = octx.enter_context(tc.tile_pool(name="const", bufs=1))
        wpool = octx.enter_context(tc.tile_pool(name="weights", bufs=1))
        mpool = octx.enter_context(tc.tile_pool(name="masks", bufs=1))
        aggpool = octx.enter_context(tc.tile_pool(name="agg", bufs=1))
        smallp = octx.enter_context(tc.tile_pool(name="small", bufs=3))
        # PSUM pools: 4 + 2 + 2 banks
        psbig = octx.enter_context(tc.tile_pool(name="psbig", bufs=5, space="PSUM"))
        pstp = octx.enter_context(tc.tile_pool(name="pstp", bufs=2, space="PSUM"))
        pssm = octx.enter_context(tc.tile_pool(name="pssm", bufs=1, space="PSUM"))

        ident = const_pool.tile([P, P], f32)
        make_identity(nc, ident[:])
        ones_f = const_pool.tile([1, P], f32)
        nc.vector.memset(ones_f[:], 1.0)
        ones_row = const_pool.tile([1, P], f32r)
        nc.scalar.copy(ones_row[:], ones_f[:])
        identr = const_pool.tile([P, P], f32r)
        nc.scalar.copy(identr[:], ident[:])

        # ---------- masks preprocessing ----------
        # transposed masks [128, 4, bpc]: chunk r, partition = token within chunk
        pmT = mpool.tile([P, 4, bpc], f32)
        pmT_term = mpool.tile([P, 4, bpc], f32)
        hmT = mpool.tile([P, 4, bpc], f32)
        with tc.tile_pool(name="mtmp", bufs=1) as mtmp:
            pm_i = mtmp.tile([bpc, L], i32)
            nc.sync.dma_start(pm_i[:], pmask[:, :])
            hm_i = mtmp.tile([bpc, L], i32)
            nc.sync.dma_start(hm_i[:], hmask[:, :])
            pm_f = mtmp.tile([bpc, L], f32)
            nc.vector.tensor_copy(pm_f[:], pm_i[:])
            hm_f = mtmp.tile([bpc, L], f32)
            nc.vector.tensor_copy(hm_f[:], hm_i[:])
            for (mf, mt, mtt) in ((pm_f, pmT, pmT_term), (hm_f, hmT, None)):
                for r in range(4):
                    pst = pstp.tile([P, HW], f32, name="pst")
                    nc.tensor.transpose(pst[:, :bpc], mf[:, r * P:(r + 1) * P],
                                        ident[:bpc, :bpc])
                    nc.scalar.copy(mt[:, r, :], pst[:, :bpc])
                    if mtt is not None:
                        nc.vector.tensor_scalar(mtt[:, r, :], pst[:, :bpc],
                                                NEG, -NEG,
                                                op0=Alu.mult, op1=Alu.add)

        # aggregate staging: [128, 8, bpc]; chunks 0-3 = premise side f-chunks
        aggT = aggpool.tile([P, 8, bpc], f32)

        # ---------- batch loop (software-pipelined: front(b) || back(b-1)) ----
        with ExitStack() as bctx:
            nat_pool = bctx.enter_context(tc.tile_pool(name="natp", bufs=1))
            embp_pool = bctx.enter_context(tc.tile_pool(name="embp", bufs=2))
            embT_pool = bctx.enter_context(tc.tile_pool(name="embT", bufs=2))
            mid_pool = bctx.enter_context(tc.tile_pool(name="mid", bufs=1))
            proj_pool = bctx.enter_context(tc.tile_pool(name="proj", bufs=1))
            att_pool = bctx.enter_context(tc.tile_pool(name="attp", bufs=2))
            al_pool = bctx.enter_context(tc.tile_pool(name="alp", bufs=1))
            trm_pool = bctx.enter_context(tc.tile_pool(name="trmp", bufs=1))

            def emit_loads(b):
                hterm_i = trm_pool.tile([1, L], i32, name="hterm_i")
                nc.sync.dma_start(hterm_i[:], hmask[b:b + 1, :])
                nat_p = nat_pool.tile([P, 4, D], f32, name="nat_p")
                nat_h = nat_pool.tile([P, 4, D], f32, name="nat_h")
                for r in range(4):
                    nc.sync.dma_start(nat_p[:, r, :], prem[b, r * P:(r + 1) * P, :])
                    nc.sync.dma_start(nat_h[:, r, :], hypo[b, r * P:(r + 1) * P, :])
                return dict(hterm_i=hterm_i, nat_p=nat_p, nat_h=nat_h)

            def emit_front(b, ld):
                hterm = trm_pool.tile([1, L], f32r, name="hterm")
                nc.vector.tensor_scalar(hterm[:], ld["hterm_i"][:], NEG, -NEG,
                                        op0=Alu.mult, op1=Alu.add)

                emb_p = embp_pool.tile([P, 4, D], f32r, name="emb_p")
                emb_h = embp_pool.tile([P, 4, D], f32r, name="emb_h")
                for r in range(4):
                    nc.vector.tensor_scalar_mul(emb_p[:, r, :], ld["nat_p"][:, r, :],
                                                pmT[:, r, b:b + 1])
                    nc.vector.tensor_scalar_mul(emb_h[:, r, :], ld["nat_h"][:, r, :],
                                                hmT[:, r, b:b + 1])
                embT_p = embT_pool.tile([P, 4, L], f32r, name="embT_p")
                embT_h = embT_pool.tile([P, 4, L], f32r, name="embT_h")
                for (src, dst) in ((emb_p, embT_p), (emb_h, embT_h)):
                    for rd in range(4):
                        pst = pstp.tile([P, HW], f32, name="pst")
                        for rc in range(4):
                            nc.tensor.transpose(
                                pst[:, rc * P:(rc + 1) * P].bitcast(f32r),
                                src[:, rc, rd * P:(rd + 1) * P],
                                identr[:])
                        nc.vector.tensor_copy(dst[:, rd, :], pst[:])

                # proj FF (attend)
                projs = []
                for (embT, nm) in ((embT_p, "projT_p"), (embT_h, "projT_h")):
                    t0T = mid_pool.tile([P, 4, L], f32r, name="midT", tag="midT")
                    for ft in range(4):
                        ps = psbig.tile([P, HW], f32, name="psb")
                        for c in range(4):
                            nc.tensor.matmul(ps[:], aw0_sb[:, c, ft * P:(ft + 1) * P],
                                             embT[:, c, :],
                                             start=(c == 0), stop=(c == 3))
                        nc.scalar.activation(t0T[:, ft, :], ps[:], Act.Relu)
                    projT = proj_pool.tile([P, 4, L], f32r, name=nm)
                    for ft in range(4):
                        ps = psbig.tile([P, HW], f32, name="psb")
                        for c in range(4):
                            nc.tensor.matmul(ps[:], aw1_sb[:, c, ft * P:(ft + 1) * P],
                                             t0T[:, c, :],
                                             start=(c == 0), stop=(c == 3))
                        nc.scalar.activation(projT[:, ft, :], ps[:], Act.Relu)
                    projs.append(projT)
                projT_p, projT_h = projs

                # att [p,h] and attT [h,p] with masking
                att = att_pool.tile([P, 4, L], f32r, name="att")
                mx = smallp.tile([P, 4], f32, name="mx")
                for pt in range(4):
                    ps = psbig.tile([P, HW], f32, name="psb")
                    for c in range(4):
                        nc.tensor.matmul(ps[:], projT_p[:, c, pt * P:(pt + 1) * P],
                                         projT_h[:, c, :],
                                         start=(c == 0), stop=False)
                    nc.tensor.matmul(ps[:], ones_row[:1, :], hterm[0:1, :],
                                     start=False, stop=True)
                    nc.scalar.activation(att[:, pt, :], ps[:], Act.Identity,
                                         bias=pmT_term[:, pt, b:b + 1])
                    nc.vector.reduce_max(mx[:, pt:pt + 1],
                                         att[:, pt, :].bitcast(f32),
                                         axis=mybir.AxisListType.X)
                # global max -> -gmax broadcast
                m1 = smallp.tile([P, 1], f32, name="m1")
                nc.vector.reduce_max(m1[:], mx[:], axis=mybir.AxisListType.X)
                pstm = pssm.tile([P, P], f32, name="pstm")
                nc.tensor.transpose(pstm[:1, :], m1[:], ident[:])
                gmax = smallp.tile([1, 1], f32, name="gmax")
                nc.vector.reduce_max(gmax[:], pstm[:1, :],
                                     axis=mybir.AxisListType.X)
                psb1 = pssm.tile([P, P], f32, name="pstm")
                nc.tensor.matmul(psb1[:, :1], ones_f[:1, :], gmax[:1, :],
                                 start=True, stop=True)
                negmax = smallp.tile([P, 1], f32, name="negmax")
                nc.scalar.mul(negmax[:], psb1[:, :1], -1.0)

                # exp (one big AP per tensor; accum_out gives Z row-sums)
                s1 = smallp.tile([P, 1], f32, name="s1")
                nc.scalar.activation(att[:, :, :], att[:, :, :], Act.Exp,
                                     bias=negmax[:, 0:1], accum_out=s1[:])
                pstz = pssm.tile([P, P], f32, name="pstm")
                nc.tensor.transpose(pstz[:1, :], s1[:], ident[:])
                zsum = smallp.tile([1, 1], f32, name="zsum")
                nc.vector.reduce_sum(zsum[:], pstz[:1, :],
                                     axis=mybir.AxisListType.X)
                rz = smallp.tile([1, 1], f32, name="rz")
                nc.vector.reciprocal(rz[:], zsum[:])
                psb2 = pssm.tile([P, P], f32, name="pstm")
                nc.tensor.matmul(psb2[:, :1], ones_f[:1, :], rz[:1, :],
                                 start=True, stop=True)
                rzb = smallp.tile([P, 1], f32, name="rzb")
                nc.scalar.copy(rzb[:], psb2[:, :1])
                return dict(b=b, emb_p=emb_p, emb_h=emb_h, embT_p=embT_p,
                            embT_h=embT_h, att=att, rzb=rzb)

            def emit_back(s):
                b = s["b"]
                # S^T via PE block transposes of the exp'd S (masking inherited)
                attT = att_pool.tile([P, 4, L], f32r, name="attT", bufs=1)
                for hi in range(4):
                    pst = pstp.tile([P, HW], f32, name="pst")
                    for pj in range(4):
                        nc.tensor.transpose(
                            pst[:, pj * P:(pj + 1) * P].bitcast(f32r),
                            s["att"][:, pj, hi * P:(hi + 1) * P],
                            identr[:])
                    nc.vector.tensor_copy(attT[:, hi, :], pst[:].bitcast(f32r))
                # alignments; 1/Z normalization fused into PSUM evacuation
                alT_p = al_pool.tile([P, 4, L], f32r, name="alT_p")
                for dt in range(4):
                    ps = psbig.tile([P, HW], f32, name="psb")
                    for c in range(4):
                        nc.tensor.matmul(ps[:],
                                         s["emb_h"][:, c, dt * P:(dt + 1) * P],
                                         attT[:, c, :],
                                         start=(c == 0), stop=(c == 3))
                    nc.vector.tensor_scalar_mul(alT_p[:, dt, :], ps[:],
                                                s["rzb"][:, 0:1])
                alT_h = al_pool.tile([P, 4, L], f32r, name="alT_h")
                for dt in range(4):
                    ps = psbig.tile([P, HW], f32, name="psb")
                    for c in range(4):
                        nc.tensor.matmul(ps[:],
                                         s["emb_p"][:, c, dt * P:(dt + 1) * P],
                                         s["att"][:, c, :],
                                         start=(c == 0), stop=(c == 3))
                    nc.vector.tensor_scalar_mul(alT_h[:, dt, :], ps[:],
                                                s["rzb"][:, 0:1])

                # compare FF + aggregate row-sums
                for (embT, alT, coff) in ((s["embT_p"], alT_p, 0),
                                          (s["embT_h"], alT_h, 4)):
                    c1T = mid_pool.tile([P, 4, L], f32r, name="midT", tag="midT")
                    for ft in range(4):
                        ps = psbig.tile([P, HW], f32, name="psb")
                        for c in range(4):
                            nc.tensor.matmul(ps[:],
                                             cw0_sb[:, c, ft * P:(ft + 1) * P],
                                             embT[:, c, :],
                                             start=(c == 0), stop=False)
                        for c in range(4):
                            nc.tensor.matmul(ps[:],
                                             cw0_sb[:, 4 + c, ft * P:(ft + 1) * P],
                                             alT[:, c, :],
                                             start=False, stop=(c == 3))
                        nc.vector.tensor_scalar_max(c1T[:, ft, :], ps[:], 0.0)
                    for ft in range(4):
                        ps = psbig.tile([P, HW], f32, name="psb")
                        for c in range(4):
                            nc.tensor.matmul(ps[:],
                                             cw1_sb[:, c, ft * P:(ft + 1) * P],
                                             c1T[:, c, :],
                                             start=(c == 0), stop=(c == 3))
                        # scratch relu output overwrites the dying alT slot
                        nc.scalar.activation(alT[:, ft, :], ps[:], Act.Relu,
                                             accum_out=aggT[:, coff + ft, b:b + 1])

            ld0 = emit_loads(0)

            # weights (after batch-0 loads so its DMAs queue first):
            # DMA to f32 staging borrowed from att_pool slots (reused by the
            # batch loop afterwards), then ACT copy rounds into f32r
            def load_w(name, src, chunks):
                t = wpool.tile([P, chunks, HW], f32r, name=name)
                for h in range(0, chunks, 4):
                    n = min(4, chunks - h)
                    stage = att_pool.tile([P, 4, HW], f32, name="att", tag="att")
                    nc.sync.dma_start(
                        stage[:, :n, :],
                        src[h * P:(h + n) * P].rearrange("(c p) f -> p c f", p=P))
                    nc.scalar.copy(t[:, h:h + n, :], stage[:, :n, :])
                return t

            aw0_sb = load_w("aw0_sb", aw0, 4)
            aw1_sb = load_w("aw1_sb", aw1, 4)
            cw0_sb = load_w("cw0_sb", cw0, 8)
            cw1_sb = load_w("cw1_sb", cw1, 4)

            prev = None
            ld = ld0
            for b in range(bpc):
                nld = emit_loads(b + 1) if b + 1 < bpc else None
                st = emit_front(b, ld)
                if prev is not None:
                    emit_back(prev)
                prev = st
                ld = nld
            emit_back(prev)

        # ---------- final aggregate FF + scorer + softmax (plain fp32) ----------
        with ExitStack() as fctx:
            fpool = fctx.enter_context(tc.tile_pool(name="fin", bufs=1))
            gw0_sb = fpool.tile([P, 8, HW], f32)
            nc.sync.dma_start(gw0_sb[:], gw0[:].rearrange("(c p) f -> p c f", p=P))
            gw1_sb = fpool.tile([P, 4, HW], f32)
            nc.sync.dma_start(gw1_sb[:], gw1[:].rearrange("(c p) f -> p c f", p=P))
            scw_sb = fpool.tile([P, 4, 2], f32)
            nc.sync.dma_start(scw_sb[:], scw[:].rearrange("(c p) s -> p c s", p=P))

            h1T = fpool.tile([P, 4, bpc], f32)
            for ft in range(4):
                ps = psbig.tile([P, HW], f32, name="psb")
                for c in range(8):
                    nc.tensor.matmul(ps[:, :bpc], gw0_sb[:, c, ft * P:(ft + 1) * P],
                                     aggT[:, c, :], start=(c == 0), stop=(c == 7))
                nc.scalar.activation(h1T[:, ft, :], ps[:, :bpc], Act.Relu)
            h2T = fpool.tile([P, 4, bpc], f32)
            for ft in range(4):
                ps = psbig.tile([P, HW], f32, name="psb")
                for c in range(4):
                    nc.tensor.matmul(ps[:, :bpc], gw1_sb[:, c, ft * P:(ft + 1) * P],
                                     h1T[:, c, :], start=(c == 0), stop=(c == 3))
                nc.scalar.activation(h2T[:, ft, :], ps[:, :bpc], Act.Relu)
            ps2 = pssm.tile([P, P], f32, name="pstm")
            for c in range(4):
                nc.tensor.matmul(ps2[:bpc, :2], h2T[:, c, :], scw_sb[:, c, :],
                                 start=(c == 0), stop=(c == 3))
            mx2 = fpool.tile([bpc, 1], f32)
            nc.vector.reduce_max(mx2[:], ps2[:bpc, :2], axis=mybir.AxisListType.X)
            negm2 = fpool.tile([bpc, 1], f32)
            nc.vector.tensor_scalar_mul(negm2[:], mx2[:], -1.0)
            prob = fpool.tile([bpc, 2], f32)
            zs2 = fpool.tile([bpc, 1], f32)
            nc.scalar.activation(prob[:], ps2[:bpc, :2], Act.Exp,
                                 bias=negm2[:, 0:1], accum_out=zs2[:])
            rz2 = fpool.tile([bpc, 1], f32)
            nc.vector.reciprocal(rz2[:], zs2[:])
            nc.vector.tensor_scalar_mul(prob[:], prob[:], rz2[:, 0:1])
            nc.sync.dma_start(out[:, :], prob[:])

    nc.compile()
    return nc


def _get(bpc):
    if bpc not in _cache:
        _cache[bpc] = _build(bpc)
    return _cache[bpc]


def kernel(premise, hypothesis, premise_mask, hypothesis_mask,
           attend_w0, attend_w1, compare_w0, compare_w1,
           aggregate_w0, aggregate_w1, scorer):
    from concourse.bass_utils import run_bass_kernel_spmd

    premise = np.asarray(premise, dtype=np.float32)
    hypothesis = np.asarray(hypothesis, dtype=np.float32)
    premise_mask = np.asarray(premise_mask, dtype=np.int32)
    hypothesis_mask = np.asarray(hypothesis_mask, dtype=np.int32)
    weights = {
        "attend_w0": np.asarray(attend_w0, dtype=np.float32),
        "attend_w1": np.asarray(attend_w1, dtype=np.float32),
        "compare_w0": np.asarray(compare_w0, dtype=np.float32),
        "compare_w1": np.asarray(compare_w1, dtype=np.float32),
        "aggregate_w0": np.asarray(aggregate_w0, dtype=np.float32),
        "aggregate_w1": np.asarray(aggregate_w1, dtype=np.float32),
        "scorer": np.asarray(scorer, dtype=np.float32),
    }
    bn = premise.shape[0]
    bpc = bn // N_CORES
    nc = _get(bpc)
    in_maps = []
    for c in range(N_CORES):
        sl = slice(c * bpc, (c + 1) * bpc)
        m = {
            "premise": np.ascontiguousarray(premise[sl]),
            "hypothesis": np.ascontiguousarray(hypothesis[sl]),
            "premise_mask": np.ascontiguousarray(premise_mask[sl]),
            "hypothesis_mask": np.ascontiguousarray(hypothesis_mask[sl]),
        }
        m.update(weights)
        in_maps.append(m)
    global _last_in_maps
    _last_in_maps = in_maps
    res = run_bass_kernel_spmd(nc, in_maps, core_ids=list(range(N_CORES)))
    return np.concatenate([res.results[c]["out"] for c in range(N_CORES)],
                          axis=0).astype(np.float32)
